# revision 3
# baseline (speedup 1.0000x reference)
"""Multi-head dense attention (no softmax) on 8 Trainium2 NeuronCores.

Math (per batch b, head h with head_dim d=64):
    q   = x @ W^T                      # [S, H] projection
    out_h = (q_h x_h^T) x_h            # naive: O(S^2 d) with an SxS temp
          = q_h (x_h^T x_h)            # reassociated: Gram matrix G_h [d, d]
The reassociation is exact (same sum, different order) and collapses the
FLOPs ~5x while removing the SxS intermediate entirely.

Sharding: core c handles batch b = c//2 and head-group hg = c%2 (8 heads,
512 output columns). Cores are fully independent (no collectives).

Device layout per core (all inputs fp16; W is pre-scaled by 1024 on the
host so its sigma~9e-5 entries clear fp16's subnormal cutoff; the Gram
tile copy multiplies by 1/1024 to undo it):
    xT  [1024, 2048]  x[b] transposed (host-prepped)  - projection operands
    xn  [2048, 512]   x[b] natural, this head-group's columns - Gram operands
    wT  [1024, 512]   1024 * W rows of this head-group, transposed (k-major)
    outT [512, 2048]  output transposed (fp16); host transposes/upcasts back

Schedule: warmup matmuls un-throttle the PE HAM clock gate during the
initial DMA window; projection is split around the DMA stream (halfA on
k-tiles 0-1, Gram on xn, halfB on k-tiles 2-7 with the out-stage one
step behind).  Inputs ride two hardware-DGE rings (Sync carries the
critical halfA prefix at full fabric bandwidth - the Scalar ring's first
descriptor is held back by a dummy dependency until the prefix lands -
then Scalar streams xn/wB/xT).  PSUM drains are spread across the
Scalar(ACT)/Vector/GpSimd engines so no single engine bottlenecks; the
out tiles are staged per m-tile and stored as single contiguous 512KB
descriptors on the Sync ring.
"""

import numpy as np

B, S, H = 4, 2048, 1024
N_HEADS = 16
HD = H // N_HEADS  # 64
N_CORES = 8
MG = H // 2        # 512 output columns per core
P = 128
KT = H // P        # 8 k-tiles
ST = S // P        # 16 s-tiles
MT = MG // P       # 4 m-tiles == head pairs
SC = S // 512      # 4 s-chunks
W_SCALE = 1024.0
KT_A = 2             # k-tiles in the first projection half
N_WARM = 5           # PE warmup matmuls (HAM un-throttle)

_NC_CACHE = {}


def _build_nc():
    import concourse.mybir as mybir
    from concourse import bacc
    from concourse.tile import TileContext

    f32 = mybir.dt.float32
    f16 = mybir.dt.float16

    nc = bacc.Bacc()
    xT_d = nc.declare_dram_parameter("xT", [H, S], f16, isOutput=False)
    xn_d = nc.declare_dram_parameter("xn", [S, MG], f16, isOutput=False)
    wT_d = nc.declare_dram_parameter("wT", [MT * P, KT * P], f16, isOutput=False)
    outT_d = nc.declare_dram_parameter("outT", [MG, S], f16, isOutput=True)

    xT_t = xT_d.rearrange("(kt p) s -> p kt s", p=P)   # [128, 8, 2048]
    xn_t = xn_d.rearrange("(st p) m -> p st m", p=P)   # [128, 16, 512]
    wT_t = wT_d.rearrange("(mt p) (kt m) -> p mt kt m", p=P, m=P)  # [128, 4, 8, 128]

    with TileContext(nc) as tc:
        with (
            tc.tile_pool(name="big", bufs=1) as big,
            tc.tile_pool(name="gp", bufs=1) as gpool,
            tc.tile_pool(name="stage", bufs=2) as stage,
        ):
            xT_sb = big.tile([P, KT, S], f16, tag="xT")
            xn_sb = big.tile([P, ST, MG], f16, tag="xn")
            wT_sb = big.tile([P, MT, KT, P], f16, tag="wT")
            qA_sb = big.tile([P, MT, S], f16, tag="qA")
            qB_sb = big.tile([P, MT, S], f16, tag="qB")
            dum_sb = big.tile([P, 512], f16, tag="dum")
            junk_sb = big.tile([P, 4], f16, tag="junk")

            # ---- PE warmup: HAM un-throttles after ~3.4us of sustained
            # activity; burn the initial DMA-wait window on dummy matmuls so
            # the real matmuls run at 2.4 GHz from the start.
            nc.gpsimd.memset(dum_sb, 0.0)
            with tc.tile_pool(name="ps_w", bufs=1, space="PSUM") as ps_w:
                psw = ps_w.tile([P, 512], f32, tag="psw")
                for i in range(N_WARM):
                    nc.tensor.matmul(
                        psw, lhsT=dum_sb[:, 0:P], rhs=dum_sb,
                        start=True, stop=True,
                    )

            # ---- Input DMA.  Sync ring: the halfA-critical prefix (all
            # m-tiles of w k-tiles 0-1, then xT k-tiles 0-1).  Scalar ring:
            # everything else, held back behind the wA landing by a dummy
            # scalar copy so the prefix gets the full fabric bandwidth.
            nc.sync.dma_start(out=wT_sb[:, :, 0:KT_A], in_=wT_t[:, :, 0:KT_A])
            for kt in range(KT_A):
                nc.sync.dma_start(out=xT_sb[:, kt], in_=xT_t[:, kt])

            nc.scalar.copy(out=junk_sb, in_=wT_sb[0:P, 0, 0, 0:4])
            nc.scalar.dma_start(out=xn_sb[:, :8], in_=xn_t[:, :8])
            nc.scalar.dma_start(out=xn_sb[:, 8:], in_=xn_t[:, 8:])
            nc.scalar.dma_start(out=wT_sb[:, :, KT_A:], in_=wT_t[:, :, KT_A:])
            for kt in range(KT_A, KT):
                nc.scalar.dma_start(out=xT_sb[:, kt], in_=xT_t[:, kt])

            # ---- First projection half: k-tiles 0-1, PSUM double-buffered
            # so the drain of m-tile i overlaps the matmuls of i+1; drains
            # alternate ACT/Vector so each engine gets a 2-period window.
            with tc.tile_pool(name="ps_qA", bufs=2, space="PSUM") as ps_qA:
                for mt in range(MT):
                    psq = ps_qA.tile([P, SC, 512], f32, tag="psq", name=f"psqA{mt}")
                    for n, kt in enumerate(range(KT_A)):
                        for sc in range(SC):
                            nc.tensor.matmul(
                                psq[:, sc],
                                lhsT=wT_sb[:, mt, kt],
                                rhs=xT_sb[:, kt, sc * 512:(sc + 1) * 512],
                                start=(n == 0),
                                stop=(n == KT_A - 1),
                            )
                    eng = nc.scalar if mt % 2 == 0 else nc.vector
                    if mt % 2 == 0:
                        nc.scalar.copy(out=qA_sb[:, mt], in_=psq)
                    else:
                        nc.vector.tensor_copy(out=qA_sb[:, mt], in_=psq)

            # ---- Gram stage: needs xn, which lands mid-stream.
            gbd = []
            with tc.tile_pool(name="ps_g", bufs=2, space="PSUM") as ps_g:
                for p_i in range(MT):
                    psg = ps_g.tile([P, P], f32, tag="psg", name=f"psg{p_i}")
                    xp = xn_sb[:, :, p_i * P:(p_i + 1) * P]
                    for i in range(ST):
                        nc.tensor.matmul(
                            psg,
                            lhsT=xp[:, i],
                            rhs=xp[:, i],
                            start=(i == 0),
                            stop=(i == ST - 1),
                        )
                    g = gpool.tile([P, P], f16, tag=f"g{p_i}", name=f"g{p_i}")
                    nc.vector.memset(g, 0.0)
                    nc.vector.tensor_scalar_mul(
                        out=g[0:HD, 0:HD], in0=psg[0:HD, 0:HD], scalar1=1.0 / W_SCALE
                    )
                    nc.vector.tensor_scalar_mul(
                        out=g[HD:P, HD:P], in0=psg[HD:P, HD:P], scalar1=1.0 / W_SCALE
                    )
                    gbd.append(g)

            with (
                tc.tile_pool(name="ps_qB", bufs=1, space="PSUM") as ps_qB,
                tc.tile_pool(name="ps_o", bufs=4, space="PSUM") as ps_o,
            ):
                # ---- Second projection half (k-tiles 2-7).  The PSUM drain
                # is a tensor_add folding qA in (q = psum + qA); the out
                # stage for chunk sc follows its drain immediately so the PE
                # only ever waits on one 684ns add.  Out tiles are cast into
                # a per-m-tile stage buffer (ACT for sc 0-1, GpSimd for sc
                # 2-3) and stored as one contiguous 512KB descriptor.
                for mt in range(MT):
                    psq = ps_qB.tile([P, SC, 512], f32, tag="psq", name=f"psqB{mt}")
                    kts_b = range(KT_A, KT)
                    for n, kt in enumerate(kts_b):
                        for sc in range(SC):
                            nc.tensor.matmul(
                                psq[:, sc],
                                lhsT=wT_sb[:, mt, kt],
                                rhs=xT_sb[:, kt, sc * 512:(sc + 1) * 512],
                                start=(n == 0),
                                stop=(n == KT - KT_A - 1),
                            )
                    ot = stage.tile([P, SC, 512], f16, tag="ot", name=f"ot{mt}")
                    for sc in range(SC):
                        nc.vector.tensor_add(
                            out=qB_sb[:, mt, sc * 512:(sc + 1) * 512],
                            in0=psq[:, sc],
                            in1=qA_sb[:, mt, sc * 512:(sc + 1) * 512],
                        )
                        pso = ps_o.tile([P, 512], f32, tag="pso", name=f"pso{mt}_{sc}")
                        nc.tensor.matmul(
                            pso,
                            lhsT=gbd[mt],
                            rhs=qB_sb[:, mt, sc * 512:(sc + 1) * 512],
                            start=True,
                            stop=True,
                        )
                        if sc < 2:
                            nc.scalar.copy(out=ot[:, sc], in_=pso)
                        else:
                            nc.vector.tensor_copy(out=ot[:, sc], in_=pso)
                    nc.sync.dma_start(
                        out=outT_d[mt * P:(mt + 1) * P, :], in_=ot
                    )
    nc.compile()
    return nc


def _get_nc():
    if "nc" not in _NC_CACHE:
        _NC_CACHE["nc"] = _build_nc()
    return _NC_CACHE["nc"]


def make_in_maps(hidden_states, queries_weight):
    hs = np.ascontiguousarray(np.asarray(hidden_states, dtype=np.float32))
    w = np.ascontiguousarray(np.asarray(queries_weight, dtype=np.float32))
    in_maps = []
    for c in range(N_CORES):
        b, hg = divmod(c, 2)
        xb = hs[b]
        in_maps.append({
            "xT": np.ascontiguousarray(xb.T).astype(np.float16),
            "xn": np.ascontiguousarray(xb[:, hg * MG:(hg + 1) * MG]).astype(
                np.float16
            ),
            "wT": np.ascontiguousarray(
                (w[hg * MG:(hg + 1) * MG, :].T * W_SCALE)
                .reshape(KT, P, MT, P)
                .transpose(2, 1, 0, 3)
                .reshape(MT * P, KT * P)
            ).astype(np.float16),
        })
    return in_maps


def assemble_output(results):
    out = np.empty((B, S, H), dtype=np.float32)
    for c in range(N_CORES):
        b, hg = divmod(c, 2)
        out[b, :, hg * MG:(hg + 1) * MG] = results[c]["outT"].T.astype(np.float32)
    return out


def kernel(hidden_states, queries_weight):
    from concourse.bass_utils import run_bass_kernel_spmd

    in_maps = make_in_maps(hidden_states, queries_weight)
    res = run_bass_kernel_spmd(
        _get_nc(), in_maps, core_ids=list(range(N_CORES))
    ).results
    return assemble_output(res)


if __name__ == "__main__":
    x = np.random.randn(B, S, H).astype(np.float32)
    w = np.random.randn(H, H).astype(np.float32) * 1e-4
    out = kernel(x, w)
    print(out.shape, out.dtype)


# revision 4
# speedup vs baseline: 1.0252x; 1.0252x over previous
"""Multi-head dense attention (no softmax) on 8 Trainium2 NeuronCores.

Math (per batch b, head h with head_dim d=64):
    q   = x @ W^T                      # [S, H] projection
    out_h = (q_h x_h^T) x_h            # naive: O(S^2 d) with an SxS temp
          = q_h (x_h^T x_h)            # reassociated: Gram matrix G_h [d, d]
The reassociation is exact (same sum, different order) and collapses the
FLOPs ~5x while removing the SxS intermediate entirely.

Sharding: core c handles batch b = c//2 and head-group hg = c%2 (8 heads,
512 output columns). Cores are fully independent (no collectives).

Device layout per core (all inputs fp16; W is pre-scaled by 1024 on the
host so its sigma~9e-5 entries clear fp16's subnormal cutoff; the Gram
tile copy multiplies by 1/1024 to undo it):
    xT  [1024, 2048]  x[b] transposed (host-prepped)  - projection operands
    xn  [2048, 512]   x[b] natural, this head-group's columns - Gram operands
    wT  [1024, 512]   1024 * W rows of this head-group, transposed (k-major)
    outT [512, 2048]  output transposed (fp16); host transposes/upcasts back

Schedule: PE-warmup matmuls un-throttle the HAM clock gate during the
initial DMA latency; the projection is split around the DMA stream
(halfA on k-tiles 0-1 while they land, Gram on xn, halfB on k-tiles 2-7
with the out stage one chunk behind).  Trn2 has exactly one fast
hardware DGE ring per direction-agnostic engine pair: the Activation
ring sustains ~320 GB/s while the SP ring and the GpSimd software ring
manage only ~45-50 GB/s.  So the Act ring carries everything
bandwidth-critical in consumption order (wA, xT 0-1, xn, xT 2-7, and the
output stores), while the two slow rings prefetch only the halfB
weights, which are small and needed late.  PSUM->SBUF drains are split
across engines: qA drains and out-tile casts on ACT, the qB fold
(tensor_add) on DVE.  Out tiles are staged per m-tile and stored as one
contiguous 512KB descriptor (the last m-tile stores per-chunk to cut
tail latency).
"""

import numpy as np

B, S, H = 4, 2048, 1024
N_HEADS = 16
HD = H // N_HEADS  # 64
N_CORES = 8
MG = H // 2        # 512 output columns per core
P = 128
KT = H // P        # 8 k-tiles
ST = S // P        # 16 s-tiles
MT = MG // P       # 4 m-tiles == head pairs
SC = S // 512      # 4 s-chunks
W_SCALE = 1024.0
KT_A = 2             # k-tiles in the first projection half
N_WARM = 5           # PE warmup matmuls (HAM un-throttle)

_NC_CACHE = {}


def _build_nc():
    import concourse.mybir as mybir
    from concourse import bacc
    from concourse.tile import TileContext

    f32 = mybir.dt.float32
    f16 = mybir.dt.float16

    nc = bacc.Bacc()
    xT_d = nc.declare_dram_parameter("xT", [H, S], f16, isOutput=False)
    xn_d = nc.declare_dram_parameter("xn", [S, MG], f16, isOutput=False)
    wT_d = nc.declare_dram_parameter("wT", [MT * P, KT * P], f16, isOutput=False)
    outT_d = nc.declare_dram_parameter("outT", [MG, S], f16, isOutput=True)

    xT_t = xT_d.rearrange("(kt p) s -> p kt s", p=P)   # [128, 8, 2048]
    xn_t = xn_d.rearrange("(st p) m -> p st m", p=P)   # [128, 16, 512]
    wT_t = wT_d.rearrange("(mt p) (kt m) -> p mt kt m", p=P, m=P)  # [128, 4, 8, 128]

    with TileContext(nc) as tc:
        with (
            tc.tile_pool(name="big", bufs=1) as big,
            tc.tile_pool(name="gp", bufs=1) as gpool,
            tc.tile_pool(name="stage", bufs=2) as stage,
        ):
            xT_sb = big.tile([P, KT, S], f16, tag="xT")
            xn_sb = big.tile([P, ST, MG], f16, tag="xn")
            wT_sb = big.tile([P, MT, KT, P], f16, tag="wT")
            qA_sb = big.tile([P, MT, S], f16, tag="qA")
            qB_sb = big.tile([P, MT, S], f16, tag="qB")
            dum_sb = big.tile([P, 512], f16, tag="dum")

            # ---- PE warmup during the DMA head.
            nc.gpsimd.memset(dum_sb, 0.0)
            with tc.tile_pool(name="ps_w", bufs=1, space="PSUM") as ps_w:
                psw = ps_w.tile([P, 512], f32, tag="psw")
                for i in range(N_WARM):
                    nc.tensor.matmul(
                        psw, lhsT=dum_sb[:, 0:P], rhs=dum_sb,
                        start=True, stop=True,
                    )

            # ---- Fast (Act) ring: consumption-order stream.
            nc.scalar.dma_start(out=wT_sb[:, 0, 0:KT_A], in_=wT_t[:, 0, 0:KT_A])
            nc.scalar.dma_start(out=wT_sb[:, 1:, 0:KT_A], in_=wT_t[:, 1:, 0:KT_A])
            for kt in range(KT_A):
                nc.scalar.dma_start(out=xT_sb[:, kt], in_=xT_t[:, kt])
            nc.scalar.dma_start(out=xn_sb[:, :8], in_=xn_t[:, :8])
            nc.scalar.dma_start(out=xn_sb[:, 8:], in_=xn_t[:, 8:])
            for kt in range(KT_A, KT):
                nc.scalar.dma_start(out=xT_sb[:, kt], in_=xT_t[:, kt])
            # ---- Slow rings prefetch the halfB weights (needed ~14us in).
            nc.sync.dma_start(out=wT_sb[:, :, KT_A:5], in_=wT_t[:, :, KT_A:5])
            nc.gpsimd.dma_start(out=wT_sb[:, :, 5:], in_=wT_t[:, :, 5:])

            # ---- First projection half: k-tiles 0-1, double-buffered PSUM;
            # batched [128,2048] drains alternate ACT/DVE.
            with tc.tile_pool(name="ps_qA", bufs=2, space="PSUM") as ps_qA:
                for mt in range(MT):
                    psq = ps_qA.tile([P, SC, 512], f32, tag="psq", name=f"psqA{mt}")
                    for n, kt in enumerate(range(KT_A)):
                        for sc in range(SC):
                            nc.tensor.matmul(
                                psq[:, sc],
                                lhsT=wT_sb[:, mt, kt],
                                rhs=xT_sb[:, kt, sc * 512:(sc + 1) * 512],
                                start=(n == 0),
                                stop=(n == KT_A - 1),
                            )
                    if mt % 2 == 0:
                        nc.scalar.copy(out=qA_sb[:, mt], in_=psq)
                    else:
                        nc.vector.tensor_copy(out=qA_sb[:, mt], in_=psq)

            # ---- Gram stage: needs xn, which lands mid-stream.
            gbd = []
            with tc.tile_pool(name="ps_g", bufs=2, space="PSUM") as ps_g:
                for p_i in range(MT):
                    psg = ps_g.tile([P, P], f32, tag="psg", name=f"psg{p_i}")
                    xp = xn_sb[:, :, p_i * P:(p_i + 1) * P]
                    for i in range(ST):
                        nc.tensor.matmul(
                            psg,
                            lhsT=xp[:, i],
                            rhs=xp[:, i],
                            start=(i == 0),
                            stop=(i == ST - 1),
                        )
                    g = gpool.tile([P, P], f16, tag=f"g{p_i}", name=f"g{p_i}")
                    nc.vector.memset(g, 0.0)
                    nc.vector.tensor_scalar_mul(
                        out=g[0:HD, 0:HD], in0=psg[0:HD, 0:HD], scalar1=1.0 / W_SCALE
                    )
                    nc.vector.tensor_scalar_mul(
                        out=g[HD:P, HD:P], in0=psg[HD:P, HD:P], scalar1=1.0 / W_SCALE
                    )
                    gbd.append(g)

            with (
                tc.tile_pool(name="ps_qB", bufs=1, space="PSUM") as ps_qB,
                tc.tile_pool(name="ps_o", bufs=4, space="PSUM") as ps_o,
            ):
                # ---- Second projection half (k-tiles 2-7); per chunk the
                # PSUM drain is a DVE tensor_add folding qA in, the out
                # matmul follows immediately, ACT casts it into the stage
                # tile, and the whole m-tile stores as one descriptor.
                for mt in range(MT):
                    psq = ps_qB.tile([P, SC, 512], f32, tag="psq", name=f"psqB{mt}")
                    kts_b = range(KT_A, KT)
                    for n, kt in enumerate(kts_b):
                        for sc in range(SC):
                            nc.tensor.matmul(
                                psq[:, sc],
                                lhsT=wT_sb[:, mt, kt],
                                rhs=xT_sb[:, kt, sc * 512:(sc + 1) * 512],
                                start=(n == 0),
                                stop=(n == KT - KT_A - 1),
                            )
                    ot = stage.tile([P, SC, 512], f16, tag="ot", name=f"ot{mt}")
                    for sc in range(SC):
                        nc.vector.tensor_add(
                            out=qB_sb[:, mt, sc * 512:(sc + 1) * 512],
                            in0=psq[:, sc],
                            in1=qA_sb[:, mt, sc * 512:(sc + 1) * 512],
                        )
                        pso = ps_o.tile([P, 512], f32, tag="pso", name=f"pso{mt}_{sc}")
                        nc.tensor.matmul(
                            pso,
                            lhsT=gbd[mt],
                            rhs=qB_sb[:, mt, sc * 512:(sc + 1) * 512],
                            start=True,
                            stop=True,
                        )
                        nc.scalar.copy(out=ot[:, sc], in_=pso)
                        if mt == MT - 1:
                            nc.scalar.dma_start(
                                out=outT_d[mt * P:(mt + 1) * P,
                                           sc * 512:(sc + 1) * 512],
                                in_=ot[:, sc],
                            )
                    if mt < MT - 1:
                        nc.scalar.dma_start(
                            out=outT_d[mt * P:(mt + 1) * P, :], in_=ot
                        )
    nc.compile()
    return nc


def _get_nc():
    if "nc" not in _NC_CACHE:
        _NC_CACHE["nc"] = _build_nc()
    return _NC_CACHE["nc"]


def make_in_maps(hidden_states, queries_weight):
    hs = np.ascontiguousarray(np.asarray(hidden_states, dtype=np.float32))
    w = np.ascontiguousarray(np.asarray(queries_weight, dtype=np.float32))
    in_maps = []
    for c in range(N_CORES):
        b, hg = divmod(c, 2)
        xb = hs[b]
        in_maps.append({
            "xT": np.ascontiguousarray(xb.T).astype(np.float16),
            "xn": np.ascontiguousarray(xb[:, hg * MG:(hg + 1) * MG]).astype(
                np.float16
            ),
            "wT": np.ascontiguousarray(
                (w[hg * MG:(hg + 1) * MG, :].T * W_SCALE)
                .reshape(KT, P, MT, P)
                .transpose(2, 1, 0, 3)
                .reshape(MT * P, KT * P)
            ).astype(np.float16),
        })
    return in_maps


def assemble_output(results):
    out = np.empty((B, S, H), dtype=np.float32)
    for c in range(N_CORES):
        b, hg = divmod(c, 2)
        out[b, :, hg * MG:(hg + 1) * MG] = results[c]["outT"].T.astype(np.float32)
    return out


def kernel(hidden_states, queries_weight):
    from concourse.bass_utils import run_bass_kernel_spmd

    in_maps = make_in_maps(hidden_states, queries_weight)
    res = run_bass_kernel_spmd(
        _get_nc(), in_maps, core_ids=list(range(N_CORES))
    ).results
    return assemble_output(res)


if __name__ == "__main__":
    x = np.random.randn(B, S, H).astype(np.float32)
    w = np.random.randn(H, H).astype(np.float32) * 1e-4
    out = kernel(x, w)
    print(out.shape, out.dtype)


# revision 7
# speedup vs baseline: 1.2014x; 1.1719x over previous
"""Multi-head dense attention (no softmax) on 8 Trainium2 NeuronCores.

Math (per batch b, head h with head_dim d=64):
    q   = x @ W^T                      # [S, H] projection
    out_h = (q_h x_h^T) x_h            # naive: O(S^2 d) with an SxS temp
          = q_h (x_h^T x_h)            # reassociated: Gram matrix G_h [d, d]
The reassociation is exact (same sum, different order) and collapses the
FLOPs ~5x while removing the SxS intermediate entirely.

Sharding: core c handles batch b = c//2 and head-group hg = c%2 (8 heads,
512 output columns). Cores are fully independent (no collectives).

Device layout per core (all inputs fp16; W is pre-scaled by 1024 on the
host so its sigma~9e-5 entries clear fp16's subnormal cutoff; the Gram
tile copy multiplies by 1/1024 to undo it):
    xT  [1024, 2048]  x[b] transposed (host-prepped)  - projection operands
    xn  [2048, 512]   x[b] natural, this head-group's columns - Gram operands
    wT  [1024, 512]   1024 * W rows of this head-group, transposed (k-major)
    outT [512, 2048]  output transposed (fp16); host transposes/upcasts back

Schedule: PE-warmup matmuls un-throttle the HAM clock gate during the
initial DMA latency; the projection is split around the DMA stream
(halfA on k-tiles 0-1 while they land, Gram on xn, halfB on k-tiles 2-7
with the out stage one chunk behind).  Trn2 has exactly one fast
hardware DGE ring per direction-agnostic engine pair: the Activation
ring sustains ~320 GB/s while the SP ring and the GpSimd software ring
manage only ~45-50 GB/s.  So the Act ring carries everything
bandwidth-critical in consumption order (wA, xT 0-1, xn, xT 2-7, and the
output stores), while the two slow rings prefetch only the halfB
weights, which are small and needed late.  PSUM->SBUF drains are split
across engines: qA drains and out-tile casts on ACT, the qB fold
(tensor_add) on DVE.  Out tiles are staged per m-tile and stored as one
contiguous 512KB descriptor (the last m-tile stores per-chunk to cut
tail latency).
"""

import numpy as np

B, S, H = 4, 2048, 1024
N_HEADS = 16
HD = H // N_HEADS  # 64
N_CORES = 8
MG = H // 2        # 512 output columns per core
P = 128
KT = H // P        # 8 k-tiles
ST = S // P        # 16 s-tiles
MT = MG // P       # 4 m-tiles == head pairs
SC = S // 512      # 4 s-chunks
W_SCALE = 1024.0
KT_A = 2             # k-tiles in the first projection half
N_WARM = 5           # PE warmup matmuls (HAM un-throttle)

_NC_CACHE = {}


def _build_nc():
    import concourse.mybir as mybir
    from concourse import bacc
    from concourse.tile import TileContext

    f32 = mybir.dt.float32
    f16 = mybir.dt.float16

    nc = bacc.Bacc()
    xT_d = nc.declare_dram_parameter("xT", [H, S], f16, isOutput=False)
    xn_d = nc.declare_dram_parameter("xn", [S, MG], f16, isOutput=False)
    # p-major weight blobs: every DMA packet is a full contiguous per-
    # partition row (2KB / 6KB) - strided slices of a single wT blob gave
    # 256-512B packets and multi-us descriptor-issue instructions.
    wA_d = nc.declare_dram_parameter("wA", [P, MT * KT_A * P], f16, isOutput=False)
    wB_d = nc.declare_dram_parameter(
        "wB", [P, MT * (KT - KT_A) * P], f16, isOutput=False
    )
    outT_d = nc.declare_dram_parameter("outT", [MG, S], f16, isOutput=True)

    xT_t = xT_d.rearrange("(kt p) s -> p kt s", p=P)   # [128, 8, 2048]
    xn_t = xn_d.rearrange("(st p) m -> p st m", p=P)   # [128, 16, 512]
    wA_t = wA_d.rearrange("p (mt kt m) -> p mt kt m", mt=MT, kt=KT_A)
    wB_t = wB_d.rearrange("p (mt kt m) -> p mt kt m", mt=MT, kt=KT - KT_A)

    with TileContext(nc) as tc:
        with (
            tc.tile_pool(name="big", bufs=1) as big,
            tc.tile_pool(name="gp", bufs=1) as gpool,
            tc.tile_pool(name="stage", bufs=2) as stage,
        ):
            xT_sb = big.tile([P, KT, S], f16, tag="xT")
            xn_sb = big.tile([P, ST, MG], f16, tag="xn")
            wT_sb = big.tile([P, MT, KT, P], f16, tag="wT")
            qA_sb = big.tile([P, MT, S], f16, tag="qA")
            qB_sb = big.tile([P, MT, S], f16, tag="qB")
            dum_sb = big.tile([P, 512], f16, tag="dum")

            # ---- PE warmup during the DMA head.
            nc.gpsimd.memset(dum_sb, 0.0)
            with tc.tile_pool(name="ps_w", bufs=1, space="PSUM") as ps_w:
                psw = ps_w.tile([P, 512], f32, tag="psw")
                for i in range(N_WARM):
                    nc.tensor.matmul(
                        psw, lhsT=dum_sb[:, 0:P], rhs=dum_sb,
                        start=True, stop=True,
                    )

            # ---- Input streams, pinned to the front of each engine's queue
            # (high_priority) so the tile scheduler can never push a dma
            # issue behind a PE-gated copy (v3 lost 23us to exactly that).
            # Sync engine/ring: the 4MB xT stream, in consumption order; it
            # has nothing else to do all kernel.  Act ring: wA + xn (the
            # halfA/Gram prefix) and later the stores.  GpSimd ring: wB.
            with tc.high_priority():
                nc.scalar.dma_start(out=wT_sb[:, :, 0:KT_A], in_=wA_t)
                for kt in range(KT):
                    nc.sync.dma_start(out=xT_sb[:, kt], in_=xT_t[:, kt])
                nc.scalar.dma_start(out=xn_sb[:, :8], in_=xn_t[:, :8])
                nc.scalar.dma_start(out=xn_sb[:, 8:], in_=xn_t[:, 8:])
                nc.gpsimd.dma_start(out=wT_sb[:, :, KT_A:], in_=wB_t)

            # ---- First projection half: k-tiles 0-1, double-buffered PSUM;
            # batched [128,2048] drains alternate ACT/DVE.
            with tc.tile_pool(name="ps_qA", bufs=2, space="PSUM") as ps_qA:
                for mt in range(MT):
                    psq = ps_qA.tile([P, SC, 512], f32, tag="psq", name=f"psqA{mt}")
                    for n, kt in enumerate(range(KT_A)):
                        for sc in range(SC):
                            nc.tensor.matmul(
                                psq[:, sc],
                                lhsT=wT_sb[:, mt, kt],
                                rhs=xT_sb[:, kt, sc * 512:(sc + 1) * 512],
                                start=(n == 0),
                                stop=(n == KT_A - 1),
                            )
                    if mt % 2 == 0:
                        nc.scalar.copy(out=qA_sb[:, mt], in_=psq)
                    else:
                        nc.vector.tensor_copy(out=qA_sb[:, mt], in_=psq)

            # ---- Gram stage: needs xn, which lands mid-stream.
            gbd = []
            with tc.tile_pool(name="ps_g", bufs=2, space="PSUM") as ps_g:
                for p_i in range(MT):
                    psg = ps_g.tile([P, P], f32, tag="psg", name=f"psg{p_i}")
                    xp = xn_sb[:, :, p_i * P:(p_i + 1) * P]
                    for i in range(ST):
                        nc.tensor.matmul(
                            psg,
                            lhsT=xp[:, i],
                            rhs=xp[:, i],
                            start=(i == 0),
                            stop=(i == ST - 1),
                        )
                    g = gpool.tile([P, P], f16, tag=f"g{p_i}", name=f"g{p_i}")
                    nc.vector.memset(g, 0.0)
                    nc.vector.tensor_scalar_mul(
                        out=g[0:HD, 0:HD], in0=psg[0:HD, 0:HD], scalar1=1.0 / W_SCALE
                    )
                    nc.vector.tensor_scalar_mul(
                        out=g[HD:P, HD:P], in0=psg[HD:P, HD:P], scalar1=1.0 / W_SCALE
                    )
                    gbd.append(g)

            with (
                tc.tile_pool(name="ps_qB", bufs=1, space="PSUM") as ps_qB,
                tc.tile_pool(name="ps_o", bufs=4, space="PSUM") as ps_o,
            ):
                # ---- Second projection half (k-tiles 2-7); per chunk the
                # PSUM drain is a DVE tensor_add folding qA in, the out
                # matmul follows immediately, ACT casts it into the stage
                # tile, and the whole m-tile stores as one descriptor.
                for mt in range(MT):
                    psq = ps_qB.tile([P, SC, 512], f32, tag="psq", name=f"psqB{mt}")
                    kts_b = range(KT_A, KT)
                    for n, kt in enumerate(kts_b):
                        for sc in range(SC):
                            nc.tensor.matmul(
                                psq[:, sc],
                                lhsT=wT_sb[:, mt, kt],
                                rhs=xT_sb[:, kt, sc * 512:(sc + 1) * 512],
                                start=(n == 0),
                                stop=(n == KT - KT_A - 1),
                            )
                    ot = stage.tile([P, SC, 512], f16, tag="ot", name=f"ot{mt}")
                    for sc in range(SC):
                        nc.vector.tensor_add(
                            out=qB_sb[:, mt, sc * 512:(sc + 1) * 512],
                            in0=psq[:, sc],
                            in1=qA_sb[:, mt, sc * 512:(sc + 1) * 512],
                        )
                        pso = ps_o.tile([P, 512], f32, tag="pso", name=f"pso{mt}_{sc}")
                        nc.tensor.matmul(
                            pso,
                            lhsT=gbd[mt],
                            rhs=qB_sb[:, mt, sc * 512:(sc + 1) * 512],
                            start=True,
                            stop=True,
                        )
                        nc.scalar.copy(out=ot[:, sc], in_=pso)
                        if mt == MT - 1:
                            nc.scalar.dma_start(
                                out=outT_d[mt * P:(mt + 1) * P,
                                           sc * 512:(sc + 1) * 512],
                                in_=ot[:, sc],
                            )
                    if mt < MT - 1:
                        nc.scalar.dma_start(
                            out=outT_d[mt * P:(mt + 1) * P, :], in_=ot
                        )
    nc.compile()
    return nc


def _get_nc():
    if "nc" not in _NC_CACHE:
        _NC_CACHE["nc"] = _build_nc()
    return _NC_CACHE["nc"]


def make_in_maps(hidden_states, queries_weight):
    hs = np.ascontiguousarray(np.asarray(hidden_states, dtype=np.float32))
    w = np.ascontiguousarray(np.asarray(queries_weight, dtype=np.float32))
    in_maps = []
    for c in range(N_CORES):
        b, hg = divmod(c, 2)
        xb = hs[b]
        # w-slice for this head group, k-major: [KT*P(k), MG(m)] ->
        # p-major blobs [p, mt, kt, m] so each DMA packet is one full
        # contiguous per-partition row.
        wk = (w[hg * MG:(hg + 1) * MG, :].T * W_SCALE).reshape(KT, P, MT, P)
        wp = wk.transpose(1, 2, 0, 3)  # [p, mt, kt, m]
        in_maps.append({
            "xT": np.ascontiguousarray(xb.T).astype(np.float16),
            "xn": np.ascontiguousarray(xb[:, hg * MG:(hg + 1) * MG]).astype(
                np.float16
            ),
            "wA": np.ascontiguousarray(
                wp[:, :, :KT_A].reshape(P, MT * KT_A * P)
            ).astype(np.float16),
            "wB": np.ascontiguousarray(
                wp[:, :, KT_A:].reshape(P, MT * (KT - KT_A) * P)
            ).astype(np.float16),
        })
    return in_maps


def assemble_output(results):
    out = np.empty((B, S, H), dtype=np.float32)
    for c in range(N_CORES):
        b, hg = divmod(c, 2)
        out[b, :, hg * MG:(hg + 1) * MG] = results[c]["outT"].T.astype(np.float32)
    return out


def kernel(hidden_states, queries_weight):
    from concourse.bass_utils import run_bass_kernel_spmd

    in_maps = make_in_maps(hidden_states, queries_weight)
    res = run_bass_kernel_spmd(
        _get_nc(), in_maps, core_ids=list(range(N_CORES))
    ).results
    return assemble_output(res)


if __name__ == "__main__":
    x = np.random.randn(B, S, H).astype(np.float32)
    w = np.random.randn(H, H).astype(np.float32) * 1e-4
    out = kernel(x, w)
    print(out.shape, out.dtype)


# revision 12
# speedup vs baseline: 1.5250x; 1.2693x over previous
"""Multi-head dense attention (no softmax) on 8 Trainium2 NeuronCores.

Math (per batch b, head h with head_dim d=64):
    q   = x @ W^T                      # [S, H] projection
    out_h = (q_h x_h^T) x_h            # naive: O(S^2 d) with an SxS temp
          = q_h (x_h^T x_h)            # reassociated: Gram matrix G_h [d, d]
The reassociation is exact (same sum, different order) and collapses the
FLOPs ~5x while removing the SxS intermediate entirely.

Sharding: core c handles batch b = c//2 and head-group hg = c%2 (8 heads,
512 output columns). Cores are fully independent (no collectives).

Device layout per core (all inputs fp16; W is pre-scaled by 1024 on the
host so its sigma~9e-5 entries clear fp16's subnormal cutoff; the Gram
tile copy multiplies by 1/1024 to undo it):
    xT  [1024, 2048]  x[b] transposed (host-prepped)  - projection operands
    xn  [2048, 512]   x[b] natural, this head-group's columns - Gram operands
    wT  [1024, 512]   1024 * W rows of this head-group, transposed (k-major)
    outT [512, 2048]  output transposed (fp16); host transposes/upcasts back

Schedule: PE-warmup matmuls un-throttle the HAM clock gate during the
initial DMA latency; the projection is split around the DMA stream
(halfA on k-tiles 0-1 while they land, Gram on xn, halfB on k-tiles 2-7
with the out stage one chunk behind).  Trn2 has exactly one fast
hardware DGE ring per direction-agnostic engine pair: the Activation
ring sustains ~320 GB/s while the SP ring and the GpSimd software ring
manage only ~45-50 GB/s.  So the Act ring carries everything
bandwidth-critical in consumption order (wA, xT 0-1, xn, xT 2-7, and the
output stores), while the two slow rings prefetch only the halfB
weights, which are small and needed late.  PSUM->SBUF drains are split
across engines: qA drains and out-tile casts on ACT, the qB fold
(tensor_add) on DVE.  Out tiles are staged per m-tile and stored as one
contiguous 512KB descriptor (the last m-tile stores per-chunk to cut
tail latency).
"""

import numpy as np

B, S, H = 4, 2048, 1024
N_HEADS = 16
HD = H // N_HEADS  # 64
N_CORES = 8
MG = H // 2        # 512 output columns per core
P = 128
KT = H // P        # 8 k-tiles
ST = S // P        # 16 s-tiles
MT = MG // P       # 4 m-tiles == head pairs
SC = S // 512      # 4 s-chunks
W_SCALE = 1024.0
KT_A = 2             # k-tiles in the first projection half
N_WARM = 5           # PE warmup matmuls (HAM un-throttle)

_NC_CACHE = {}


def _build_nc():
    import concourse.mybir as mybir
    from concourse import bacc
    from concourse.tile import TileContext

    f32 = mybir.dt.float32
    f16 = mybir.dt.float16

    nc = bacc.Bacc()
    xT_d = nc.declare_dram_parameter("xT", [H, S], f16, isOutput=False)
    xn_d = nc.declare_dram_parameter("xn", [P, ST * MG], f16, isOutput=False)
    # p-major weight blobs: every DMA packet is a full contiguous per-
    # partition row (2KB / 6KB) - strided slices of a single wT blob gave
    # 256-512B packets and multi-us descriptor-issue instructions.
    wA_d = nc.declare_dram_parameter("wA", [P, MT * KT_A * P], f16, isOutput=False)
    wB_d = nc.declare_dram_parameter(
        "wB", [P, MT * (KT - KT_A) * P], f16, isOutput=False
    )
    outT_d = nc.declare_dram_parameter("outT", [MG, S], f16, isOutput=True)

    xT_t = xT_d.rearrange("(kt p) s -> p kt s", p=P)   # [128, 8, 2048]
    xn_t = xn_d.rearrange("p (st m) -> p st m", st=ST)  # [128, 16, 512]
    wA_t = wA_d.rearrange("p (mt kt m) -> p mt kt m", mt=MT, kt=KT_A)
    wB_t = wB_d.rearrange("p (mt kt m) -> p mt kt m", mt=MT, kt=KT - KT_A)

    with TileContext(nc) as tc:
        with (
            tc.tile_pool(name="big", bufs=1) as big,
            tc.tile_pool(name="gp", bufs=1) as gpool,
            tc.tile_pool(name="stage", bufs=2) as stage,
        ):
            xT_sb = big.tile([P, KT, S], f16, tag="xT")
            xn_sb = big.tile([P, ST, MG], f16, tag="xn")
            wT_sb = big.tile([P, MT, KT, P], f16, tag="wT")
            qA_sb = big.tile([P, MT, S], f16, tag="qA")
            qB_sb = big.tile([P, MT, S], f16, tag="qB")
            dum_sb = big.tile([P, 512], f16, tag="dum")

            # ---- PE warmup during the DMA head.
            nc.gpsimd.memset(dum_sb, 0.0)
            with tc.tile_pool(name="ps_w", bufs=1, space="PSUM") as ps_w:
                psw = ps_w.tile([P, 512], f32, tag="psw")
                for i in range(N_WARM):
                    nc.tensor.matmul(
                        psw, lhsT=dum_sb[:, 0:P], rhs=dum_sb,
                        start=True, stop=True,
                    )

            # ---- Input streams, pinned to the front of each engine's queue
            # (high_priority) so the tile scheduler can never push a dma
            # issue behind a PE-gated copy (v3 lost 23us to exactly that).
            # Sync engine/ring: the 4MB xT stream, in consumption order; it
            # has nothing else to do all kernel.  Act ring: wA + xn (the
            # halfA/Gram prefix) and later the stores.  GpSimd ring: wB.
            with tc.high_priority():
                nc.scalar.dma_start(out=wT_sb[:, :, 0:KT_A], in_=wA_t)
                for kt in range(KT):
                    nc.sync.dma_start(out=xT_sb[:, kt], in_=xT_t[:, kt])
                nc.scalar.dma_start(out=xn_sb[:, :8], in_=xn_t[:, :8])
                nc.scalar.dma_start(out=xn_sb[:, 8:], in_=xn_t[:, 8:])
                nc.gpsimd.dma_start(out=wT_sb[:, :, KT_A:], in_=wB_t)

            # ---- First projection half: k-tiles 0-1, double-buffered PSUM;
            # per-m-tile drains split across ACT and DVE so the drain
            # latency (~1.4us per half) stays under the 1.7us matmul group.
            with tc.tile_pool(name="ps_qA", bufs=2, space="PSUM") as ps_qA:
                for mt in range(MT):
                    psq = ps_qA.tile([P, SC, 512], f32, tag="psq", name=f"psqA{mt}")
                    for n, kt in enumerate(range(KT_A)):
                        for sc in range(SC):
                            nc.tensor.matmul(
                                psq[:, sc],
                                lhsT=wT_sb[:, mt, kt],
                                rhs=xT_sb[:, kt, sc * 512:(sc + 1) * 512],
                                start=(n == 0),
                                stop=(n == KT_A - 1),
                            )
                    nc.scalar.copy(
                        out=qA_sb[:, mt, 0:1024], in_=psq[:, 0:2]
                    )
                    nc.vector.tensor_copy(
                        out=qA_sb[:, mt, 1024:2048], in_=psq[:, 2:4]
                    )

            # ---- Gram stage: needs xn, which lands mid-stream.
            gbd = []
            with tc.tile_pool(name="ps_g", bufs=2, space="PSUM") as ps_g:
                for p_i in range(MT):
                    psg = ps_g.tile([P, P], f32, tag="psg", name=f"psg{p_i}")
                    xp = xn_sb[:, :, p_i * P:(p_i + 1) * P]
                    for i in range(ST):
                        nc.tensor.matmul(
                            psg,
                            lhsT=xp[:, i],
                            rhs=xp[:, i],
                            start=(i == 0),
                            stop=(i == ST - 1),
                        )
                    g = gpool.tile([P, P], f16, tag=f"g{p_i}", name=f"g{p_i}")
                    nc.vector.memset(g, 0.0)
                    nc.vector.tensor_scalar_mul(
                        out=g[0:HD, 0:HD], in0=psg[0:HD, 0:HD], scalar1=1.0 / W_SCALE
                    )
                    nc.vector.tensor_scalar_mul(
                        out=g[HD:P, HD:P], in0=psg[HD:P, HD:P], scalar1=1.0 / W_SCALE
                    )
                    gbd.append(g)

            with (
                tc.tile_pool(name="ps_qB", bufs=2, space="PSUM") as ps_qB,
                tc.tile_pool(name="ps_o", bufs=4, space="PSUM") as ps_o,
            ):
                # ---- Second projection half (k-tiles 2-7) as a pipeline of
                # half-m-tile passes (sc-pairs, 2 PSUM banks each).  The DVE
                # tensor_add folding qA in (q = psum + qA) and the out
                # matmuls for pass i are emitted behind pass i+1's
                # projection matmuls, so the PE never waits on a drain: by
                # the time the out matmul issues, its ~2.6us-older fold is
                # long done.  ACT casts out tiles; each m-tile stores as one
                # contiguous 512KB descriptor (last one per-chunk to cut
                # tail latency).
                ots = [
                    stage.tile([P, SC, 512], f16, tag="ot", name=f"ot{mt}")
                    for mt in range(2)
                ]
                passes = [(mt, h) for mt in range(MT) for h in range(2)]

                def proj_pass(mt, h):
                    psq = ps_qB.tile(
                        [P, 2, 512], f32, tag="psq", name=f"psqB{mt}_{h}"
                    )
                    for n, kt in enumerate(range(KT_A, KT)):
                        for sc in (2 * h, 2 * h + 1):
                            nc.tensor.matmul(
                                psq[:, sc - 2 * h],
                                lhsT=wT_sb[:, mt, kt],
                                rhs=xT_sb[:, kt, sc * 512:(sc + 1) * 512],
                                start=(n == 0),
                                stop=(n == KT - KT_A - 1),
                            )
                    for sc in (2 * h, 2 * h + 1):
                        nc.vector.tensor_add(
                            out=qB_sb[:, mt, sc * 512:(sc + 1) * 512],
                            in0=psq[:, sc - 2 * h],
                            in1=qA_sb[:, mt, sc * 512:(sc + 1) * 512],
                        )

                def out_pass(mt, h):
                    ot = ots[mt % 2]
                    for sc in (2 * h, 2 * h + 1):
                        pso = ps_o.tile([P, 512], f32, tag="pso", name=f"pso{mt}_{sc}")
                        nc.tensor.matmul(
                            pso,
                            lhsT=gbd[mt],
                            rhs=qB_sb[:, mt, sc * 512:(sc + 1) * 512],
                            start=True,
                            stop=True,
                        )
                        nc.scalar.copy(out=ot[:, sc], in_=pso)
                        if mt == MT - 1:
                            nc.scalar.dma_start(
                                out=outT_d[mt * P:(mt + 1) * P,
                                           sc * 512:(sc + 1) * 512],
                                in_=ot[:, sc],
                            )
                    if mt < MT - 1 and h == 1:
                        nc.scalar.dma_start(
                            out=outT_d[mt * P:(mt + 1) * P, :], in_=ot
                        )

                for n, (mt, h) in enumerate(passes):
                    proj_pass(mt, h)
                    if n > 0:
                        out_pass(*passes[n - 1])
                out_pass(*passes[-1])
    nc.compile()
    return nc


def _get_nc():
    if "nc" not in _NC_CACHE:
        _NC_CACHE["nc"] = _build_nc()
    return _NC_CACHE["nc"]


def make_in_maps(hidden_states, queries_weight):
    hs = np.ascontiguousarray(np.asarray(hidden_states, dtype=np.float32))
    w = np.ascontiguousarray(np.asarray(queries_weight, dtype=np.float32))
    in_maps = []
    for c in range(N_CORES):
        b, hg = divmod(c, 2)
        xb = hs[b]
        # w-slice for this head group, k-major: [KT*P(k), MG(m)] ->
        # p-major blobs [p, mt, kt, m] so each DMA packet is one full
        # contiguous per-partition row.
        wk = (w[hg * MG:(hg + 1) * MG, :].T * W_SCALE).reshape(KT, P, MT, P)
        wp = wk.transpose(1, 2, 0, 3)  # [p, mt, kt, m]
        xnb = (
            xb[:, hg * MG:(hg + 1) * MG]
            .reshape(ST, P, MG)
            .transpose(1, 0, 2)
            .reshape(P, ST * MG)
        )
        in_maps.append({
            "xT": np.ascontiguousarray(xb.T).astype(np.float16),
            "xn": np.ascontiguousarray(xnb).astype(np.float16),
            "wA": np.ascontiguousarray(
                wp[:, :, :KT_A].reshape(P, MT * KT_A * P)
            ).astype(np.float16),
            "wB": np.ascontiguousarray(
                wp[:, :, KT_A:].reshape(P, MT * (KT - KT_A) * P)
            ).astype(np.float16),
        })
    return in_maps


def assemble_output(results):
    out = np.empty((B, S, H), dtype=np.float32)
    for c in range(N_CORES):
        b, hg = divmod(c, 2)
        out[b, :, hg * MG:(hg + 1) * MG] = results[c]["outT"].T.astype(np.float32)
    return out


def kernel(hidden_states, queries_weight):
    from concourse.bass_utils import run_bass_kernel_spmd

    in_maps = make_in_maps(hidden_states, queries_weight)
    res = run_bass_kernel_spmd(
        _get_nc(), in_maps, core_ids=list(range(N_CORES))
    ).results
    return assemble_output(res)


if __name__ == "__main__":
    x = np.random.randn(B, S, H).astype(np.float32)
    w = np.random.randn(H, H).astype(np.float32) * 1e-4
    out = kernel(x, w)
    print(out.shape, out.dtype)
